# revision 1
# baseline (speedup 1.0000x reference)
"""CRF NLL loss kernel for 8 Trainium2 NeuronCores (Bass/Tile).

Strategy (data-parallel, batch sharded 32 per core):
  - Forward algorithm runs in the exp domain: E_t = g_t * (A @ E_{t-1}) with
    A = exp(T) (bf16) and g_t = exp(feat_t - C0).  Each step is one PE matmul
    [128x128 @ 128x32] plus one DVE tensor_tensor (the PSUM->SBUF mover).
  - The 1023-step serial chain is halved by meeting in the middle: a forward
    chain (alpha) and a backward chain (beta) run concurrently on each core,
    511/512 rounds each; logZ_b = log(sum_j Ef[j,b]*Eb[j,b]) + carries.
  - Periodic renormalization (every RK rounds) keeps magnitudes in f32/bf16
    range; the applied reciprocal scales are recorded and the log-carries are
    applied on the host in f64.  The renorm multiply is folded into the g
    slice off the critical path.
  - feats are streamed once from HBM, exponentiated on the ACT engine into a
    t-major bf16 DRAM scratch, then DMA-xbar-transposed into SBUF as
    g[c=128, t*32+b] so chain movers read contiguous 32-column slices.
  - The gold (numerator) score is computed with GPSIMD index math + indirect
    DMA gathers (emissions from feats, transitions from T), reduced on chip.
  - Host does only the O(B) final combine in f64.
"""

import os
import numpy as np

GOLD_MODE = os.environ.get("CRF_GOLD", "full")   # full | nomask | nomm | noext | off

B, L, C = 256, 1024, 128
NCORES = 8
BC = B // NCORES          # 32 sequences per core
M = (L - 1) // 2          # 511: meet point, alpha_M vs beta_M
C0 = 5.3                  # static per-step log-scale absorbed into g
RK = 64                   # renorm every RK chain rounds
RLAG = 4                  # renorm scale measured RLAG rounds early
FWD_REN = [t for t in range(1, M + 1) if t % RK == 0 and t - RLAG >= 1]
BWD_REN = [t for t in range(L - 1, M + 1, -1) if t % RK == 0 and t + RLAG <= L - 1]
NRF, NRB = len(FWD_REN), len(BWD_REN)
TCH = 128                 # time steps per stream chunk
NCH = L // TCH            # 16 chunks of 2048 scratch rows

_cache = {}


def _build():
    import concourse.bacc as bacc
    import concourse.mybir as mybir
    import concourse.bass as bass
    from concourse.tile import TileContext

    f32 = mybir.dt.float32
    bf16 = mybir.dt.bfloat16
    i32 = mybir.dt.int32
    MUL = mybir.AluOpType.mult
    ADD = mybir.AluOpType.add

    nc = bacc.Bacc("TRN2")
    feats = nc.dram_tensor("feats", [BC, L, C], f32, kind="ExternalInput")
    tags = nc.dram_tensor("tags", [BC, L], i32, kind="ExternalInput")
    Tm = nc.dram_tensor("T", [C, C], f32, kind="ExternalInput")
    ef_o = nc.dram_tensor("ef", [C, BC], f32, kind="ExternalOutput")
    eb_o = nc.dram_tensor("eb", [C, BC], f32, kind="ExternalOutput")
    recf_o = nc.dram_tensor("recf", [max(NRF, 1), BC], f32, kind="ExternalOutput")
    recb_o = nc.dram_tensor("recb", [max(NRB, 1), BC], f32, kind="ExternalOutput")
    gold_o = nc.dram_tensor("gold", [C, 2], f32, kind="ExternalOutput")

    with TileContext(nc) as tc:
        with (
            tc.tile_pool(name="const", bufs=1) as cp,
            tc.tile_pool(name="gbig", bufs=1) as gp,
            tc.tile_pool(name="stage", bufs=2) as sp,
            tc.tile_pool(name="state", bufs=1) as st,
            tc.tile_pool(name="small", bufs=2) as sm,
            tc.tile_pool(name="dram", bufs=1, space="DRAM") as dp,
            tc.tile_pool(name="psum", bufs=2, space="PSUM") as pp,
        ):
            # ---- constants: A = exp(T) bf16 (natural [j,k]) and its transpose
            t_f32 = cp.tile([C, C], f32, tag="t_f32")
            nc.sync.dma_start(t_f32[:], Tm[:])
            AB = cp.tile([C, C], bf16, tag="AB")
            nc.scalar.activation(AB[:], t_f32[:], mybir.ActivationFunctionType.Exp)
            AF = cp.tile([C, C], bf16, tag="AF")
            nc.sync.dma_start_transpose(AF[:], AB[:])
            ones_col = cp.tile([C, 1], bf16, tag="ones_col")
            nc.vector.memset(ones_col[:], 1.0)
            ones_row = cp.tile([1, C], bf16, tag="ones_row")
            nc.vector.memset(ones_row[:], 1.0)
            biasc = cp.tile([128, 1], f32, tag="biasc")
            nc.vector.memset(biasc[:], -C0)

            # ---- gold score setup (one-hot matmul scheme; no indirect DMA)
            # iota constants and identity
            iota_i = cp.tile([C, C], i32, tag="iota_i")
            nc.gpsimd.iota(iota_i[:], pattern=[[1, C]], base=0, channel_multiplier=0)
            iota_c = cp.tile([C, C], bf16, tag="iota_c")
            nc.gpsimd.tensor_copy(iota_c[:], iota_i[:])
            iop_i = cp.tile([C, 1], i32, tag="iop_i")
            nc.gpsimd.iota(iop_i[:], pattern=[[0, 1]], base=0, channel_multiplier=1)
            iop_c = cp.tile([C, 1], f32, tag="iop_c")
            nc.gpsimd.tensor_copy(iop_c[:], iop_i[:])
            ident = cp.tile([C, C], bf16, tag="ident")
            nc.gpsimd.tensor_scalar(ident[:], iota_c[:], iop_c[:], None,
                                    op0=mybir.AluOpType.is_equal)
            # tags -> bf16 -> DRAM -> xbar transpose -> tgT[p, b*8+k] = tags[b, k*128+p]
            tg = cp.tile([BC, L], i32, tag="tg")
            nc.sync.dma_start(tg[:], tags[:])
            tg_bf = cp.tile([BC, L], bf16, tag="tg_bf")
            nc.gpsimd.tensor_copy(tg_bf[:], tg[:])
            tscr = dp.tile([C + BC * L, 1], bf16, tag="tscr")   # 128-elem pad at front
            nc.sync.dma_start(
                tscr[C:C + BC * L, 0:1].rearrange("(b t) o -> b (t o)", b=BC), tg_bf[:])
            NTCH = L // 128                      # 8 tag chunks per batch
            tgT_bf = cp.tile([C, BC * NTCH], bf16, tag="tgT_bf")
            nc.sync.dma_start_transpose(
                tgT_bf[:], tscr[C:C + BC * L, 0:1].rearrange("(m p) o -> m (p o)", p=C))
            tgT = cp.tile([C, BC * NTCH], f32, tag="tgT")
            nc.gpsimd.tensor_copy(tgT[:], tgT_bf[:])
            tgTs_bf = cp.tile([C, BC * NTCH], bf16, tag="tgTs_bf")
            nc.sync.dma_start_transpose(
                tgTs_bf[:], tscr[C - 1:C - 1 + BC * L, 0:1].rearrange("(m p) o -> m (p o)", p=C))
            tgTs = cp.tile([C, BC * NTCH], f32, tag="tgTs")
            nc.gpsimd.tensor_copy(tgTs[:], tgTs_bf[:])
            # held PSUM accumulators
            emacc = pp.tile([C, C], f32, name="emacc", tag="emacc", bufs=1)
            tracc = pp.tile([C, C], f32, name="tracc", tag="tracc", bufs=1)

            # ---- stream feats -> exp -> t-major bf16 scratch -> transpose to SBUF
            # scratch row r = t*BC + b;  g columns land as [c, t*BC + b]
            gnat = dp.tile([L * BC, C], bf16, tag="gnat")
            g = gp.tile([C, L * BC], bf16, tag="g")      # 64KB/partition
            RPC = TCH * BC                  # rows per chunk (4096)
            gnat_r = gnat.rearrange("(p w) c -> p (w c)", w=BC)
            order = []                      # serve both chain ends first
            lo, hi = 0, NCH - 1
            while lo <= hi:
                order.append(lo)
                if hi != lo:
                    order.append(hi)
                lo, hi = lo + 1, hi - 1
            nmm = [0]
            for k in order:
                s_in = sp.tile([128, BC * C], f32, name="s_in", tag="s_in")
                src = feats[:, k * TCH:(k + 1) * TCH, :].rearrange("b tt c -> tt b c")
                nc.sync.dma_start(s_in[:].rearrange("p (b c) -> p b c", b=BC), src)
                s_bf = sp.tile([128, BC * C], bf16, name="s_bf", tag="s_bf")
                nc.scalar.activation(s_bf[:], s_in[:],
                                     mybir.ActivationFunctionType.Exp, bias=biasc[:])
                nc.sync.dma_start(gnat_r[k * 128:(k + 1) * 128, :], s_bf[:])
                nc.sync.dma_start_transpose(
                    g[:, k * RPC:(k + 1) * RPC], gnat[k * RPC:(k + 1) * RPC, :])
                # gold contributions from this chunk (rows = t in chunk, per batch)
                s_fb = sp.tile([128, BC * C], bf16, name="s_fb", tag="s_fb")
                nc.scalar.copy(s_fb[:], s_in[:])
                for b in range(BC if GOLD_MODE != "off" else 0):
                    m = b * NTCH + k
                    oh = sm.tile([C, C], bf16, name="oh", tag="oh", bufs=4)
                    nc.gpsimd.tensor_scalar(oh[:], iota_c[:], tgT[:, m:m + 1],
                                            None, op0=mybir.AluOpType.is_equal)
                    ohp = sm.tile([C, C], bf16, name="ohp", tag="ohp", bufs=4)
                    nc.gpsimd.tensor_scalar(ohp[:], iota_c[:], tgTs[:, m:m + 1],
                                            None, op0=mybir.AluOpType.is_equal)
                    if k == 0 and GOLD_MODE not in ("nomask",):
                        nc.gpsimd.memset(ohp[0:1, :], 0.0)   # t=0 has no predecessor
                    first = nmm[0] == 0
                    last = nmm[0] == BC * NCH - 1
                    if GOLD_MODE not in ("nomm",):
                        nc.tensor.matmul(emacc[:], oh[:], s_fb[:, b * C:(b + 1) * C],
                                         start=first, stop=last, skip_group_check=True)
                        nc.tensor.matmul(tracc[:], oh[:], ohp[:],
                                         start=first, stop=last, skip_group_check=True)
                    nmm[0] += 1

            # ---- the two chains
            EBUF = 3
            Ebufs = [st.tile([C, BC], bf16, name=f"E{i}", tag=f"E{i}") for i in range(EBUF)]
            Vbufs = [st.tile([C, BC], bf16, name=f"V{i}", tag=f"V{i}") for i in range(EBUF)]
            nc.vector.tensor_copy(Ebufs[0][:], g[:, 0:BC])                     # alpha_0
            nc.vector.tensor_copy(Vbufs[0][:], g[:, (L - 1) * BC: L * BC])     # v_{L-1}

            fwd_gs = {}
            bwd_gs = {}

            def renorm_prep(src_state, gslice, rec_row, which):
                """Compute rec = 1/colsum(state), record it, scale the g slice."""
                cs = pp.tile([1, BC], f32, name="cs", tag="cs", bufs=1)
                nc.tensor.matmul(cs[:], ones_col[:], src_state[:], start=True, stop=True)
                rec = sm.tile([1, BC], bf16, name="rec_bf", tag="rec_bf")
                with nc.allow_low_precision(reason="applied scale is recorded exactly"):
                    nc.vector.reciprocal(rec[:], cs[:])
                rec_st = sm.tile([1, BC], f32, name="rec_st", tag="rec_st")
                nc.scalar.copy(rec_st[:], rec[:])
                nc.sync.dma_start(rec_row, rec_st[:])
                bc = pp.tile([C, BC], f32, name="bc", tag="bc", bufs=1)
                nc.tensor.matmul(bc[:], ones_row[:], rec[:], start=True, stop=True)
                gs = sm.tile([C, BC], bf16, name=f"gs{which}", tag=f"gs{which}")
                nc.vector.tensor_tensor(out=gs[:], in0=bc[:], in1=gslice, op=MUL)
                return gs

            fi = bi = 0
            for i in range(511):
                tf = i + 1                       # fwd round: produces alpha-state tf
                tb = L - 1 - i                   # bwd round: consumes g_{tb-1}
                if tf + RLAG in FWD_REN:
                    tr = tf + RLAG
                    fwd_gs[tr] = renorm_prep(Ebufs[i % EBUF],
                                             g[:, tr * BC:(tr + 1) * BC],
                                             recf_o[fi:fi + 1, :], "f")
                    fi += 1
                if tb - RLAG in BWD_REN:
                    tr = tb - RLAG
                    bwd_gs[tr] = renorm_prep(Vbufs[i % EBUF],
                                             g[:, (tr - 1) * BC: tr * BC],
                                             recb_o[bi:bi + 1, :], "b")
                    bi += 1
                psf = pp.tile([C, BC], f32, name="psf", tag="psf", bufs=2)
                nc.tensor.matmul(psf[:], AF[:], Ebufs[i % EBUF][:], start=True, stop=True)
                gin = fwd_gs.pop(tf, None)
                gsl = gin[:] if gin is not None else g[:, tf * BC:(tf + 1) * BC]
                nc.vector.tensor_tensor(out=Ebufs[(i + 1) % EBUF][:], in0=psf[:],
                                        in1=gsl, op=MUL)
                psb = pp.tile([C, BC], f32, name="psb", tag="psb", bufs=2)
                nc.tensor.matmul(psb[:], AB[:], Vbufs[i % EBUF][:], start=True, stop=True)
                gin = bwd_gs.pop(tb, None)
                gsl = gin[:] if gin is not None else g[:, (tb - 1) * BC: tb * BC]
                nc.vector.tensor_tensor(out=Vbufs[(i + 1) % EBUF][:], in0=psb[:],
                                        in1=gsl, op=MUL)

            # fwd state after 511 rounds = alpha~_511; bwd needs one more matmul
            ef_t = st.tile([C, BC], f32, name="ef_t", tag="ef_t")
            nc.scalar.copy(ef_t[:], Ebufs[511 % EBUF][:])
            nc.sync.dma_start(ef_o[:], ef_t[:])
            psb = pp.tile([C, BC], f32, name="psb_fin", tag="psb", bufs=2)
            nc.tensor.matmul(psb[:], AB[:], Vbufs[511 % EBUF][:], start=True, stop=True)
            eb_t = st.tile([C, BC], f32, name="eb_t", tag="eb_t")
            nc.scalar.copy(eb_t[:], psb[:])
            nc.sync.dma_start(eb_o[:], eb_t[:])
            # gold extraction: emit = trace(emacc); trans = <T, tracc>
            from concourse import bass_isa
            if GOLD_MODE in ("off", "nomm", "noext"):
                dummy = cp.tile([C, 2], f32, tag="dummy")
                nc.vector.memset(dummy[:], 0.0)
                nc.sync.dma_start(gold_o[:], dummy[:])
            extraction_on = GOLD_MODE in ("full", "nomask")
            scr0 = cp.tile([C, C], f32, tag="scr0")
            dsum = cp.tile([C, 2], f32, tag="dsum")
            if extraction_on:
                nc.vector.tensor_tensor(out=scr0[:], in0=emacc[:], in1=ident[:], op=MUL)
                nc.vector.tensor_reduce(dsum[:, 0:1], scr0[:],
                                        axis=mybir.AxisListType.X, op=ADD)
                scr1 = cp.tile([C, C], f32, tag="scr1")
                nc.vector.tensor_tensor(out=scr1[:], in0=tracc[:], in1=t_f32[:], op=MUL)
                nc.vector.tensor_reduce(dsum[:, 1:2], scr1[:],
                                        axis=mybir.AxisListType.X, op=ADD)
                nc.sync.dma_start(gold_o[:], dsum[:])

    nc.compile()
    return nc


def _get_nc():
    if "nc" not in _cache:
        _cache["nc"] = _build()
    return _cache["nc"]


def kernel(feats, tags, T, _trace=False, _trace_kwargs=None):
    from concourse.bass_utils import run_bass_kernel_spmd

    feats = np.ascontiguousarray(feats, dtype=np.float32)
    tags = np.ascontiguousarray(tags, dtype=np.int32)
    T = np.ascontiguousarray(T, dtype=np.float32)

    nc = _get_nc()
    in_maps = []
    for c in range(NCORES):
        sl = slice(c * BC, (c + 1) * BC)
        in_maps.append({"feats": feats[sl], "tags": tags[sl], "T": T})
    res = run_bass_kernel_spmd(nc, in_maps, core_ids=list(range(NCORES)),
                               trace=_trace, **(_trace_kwargs or {}))
    if _trace:
        _cache["last_results"] = res

    logZ = np.zeros(B)
    gold_total = 0.0
    for c, r in enumerate(res.results):
        sl = slice(c * BC, (c + 1) * BC)
        ef = r["ef"].astype(np.float64)
        eb = r["eb"].astype(np.float64)
        carry = 0.0
        if NRF:
            carry = carry - np.log(r["recf"].astype(np.float64)).sum(axis=0)
        if NRB:
            carry = carry - np.log(r["recb"].astype(np.float64)).sum(axis=0)
        logZ[sl] = np.log((ef * eb).sum(axis=0)) + carry + L * C0
        gold_total += float(r["gold"].astype(np.float64).sum())
    loss = logZ.mean() - gold_total / B
    return np.float32(loss)



# revision 4
# speedup vs baseline: 3.2754x; 3.2754x over previous
"""CRF NLL loss kernel for 8 Trainium2 NeuronCores (Bass/Tile).

Strategy (data-parallel, batch sharded 32 per core):
  - Forward algorithm runs in the exp domain: E_t = g_t * (A @ E_{t-1}) with
    A = exp(T) (bf16) and g_t = exp(feat_t - C0).  Each step is one PE matmul
    [128x128 @ 128x32] plus one DVE tensor_tensor (the PSUM->SBUF mover).
  - The 1023-step serial chain is halved by meeting in the middle: a forward
    chain (alpha) and a backward chain (beta) run concurrently on each core,
    511/512 rounds each; logZ_b = log(sum_j Ef[j,b]*Eb[j,b]) + carries.
  - Periodic renormalization (every RK rounds) keeps magnitudes in f32/bf16
    range; the applied reciprocal scales are recorded and the log-carries are
    applied on the host in f64.
  - feats are streamed once from HBM, exponentiated on the ACT engine into a
    t-major bf16 DRAM scratch, then DMA-xbar-transposed into SBUF as
    g[c=128, t*32+b] so chain movers read contiguous 32-column slices.
  - Gold (numerator) score without any GPSIMD work: per 128-t chunk, bulk
    one-hot tiles oh[t, (b,lab)] are built with ONE DVE is_equal using
    0-stride broadcast APs (iota vs transposed tags); emissions are one DVE
    scalar_tensor_tensor (oh * feats, accum_out=row sums); transition counts
    accumulate in a held PSUM bank via 32 PE matmuls per chunk
    (oh_cur^T @ oh_prev), dotted with T at the end.
  - Host does only the O(B) final combine in f64.
"""

import numpy as np

B, L, C = 256, 1024, 128
NCORES = 8
BC = B // NCORES          # 32 sequences per core
M = (L - 1) // 2          # 511: meet point, alpha_M vs beta_M
C0 = 5.3                  # static per-step log-scale absorbed into g
RK = 64                   # renorm every RK chain rounds
RLAG = 4                  # renorm scale measured RLAG rounds early
FWD_REN = [t for t in range(1, M + 1) if t % RK == 0 and t - RLAG >= 1]
BWD_REN = [t for t in range(L - 1, M + 1, -1) if t % RK == 0 and t + RLAG <= L - 1]
NRF, NRB = len(FWD_REN), len(BWD_REN)
TCH = 128                 # time steps per stream chunk
NCH = L // TCH            # 8 chunks
NTCH = L // 128           # 8 tag blocks of 128 t per sequence

_cache = {}


def _build():
    import concourse.bacc as bacc
    import concourse.mybir as mybir
    from concourse.tile import TileContext

    f32 = mybir.dt.float32
    bf16 = mybir.dt.bfloat16
    i32 = mybir.dt.int32
    MUL = mybir.AluOpType.mult
    EQ = mybir.AluOpType.is_equal

    nc = bacc.Bacc("TRN2")
    feats = nc.dram_tensor("feats", [BC, L, C], f32, kind="ExternalInput")
    tags = nc.dram_tensor("tags", [BC, L], i32, kind="ExternalInput")
    Tm = nc.dram_tensor("T", [C, C], f32, kind="ExternalInput")
    ef_o = nc.dram_tensor("ef", [C, BC], f32, kind="ExternalOutput")
    eb_o = nc.dram_tensor("eb", [C, BC], f32, kind="ExternalOutput")
    recf_o = nc.dram_tensor("recf", [max(NRF, 1), BC], f32, kind="ExternalOutput")
    recb_o = nc.dram_tensor("recb", [max(NRB, 1), BC], f32, kind="ExternalOutput")
    gold_o = nc.dram_tensor("gold", [C, NCH + 1], f32, kind="ExternalOutput")

    with TileContext(nc) as tc:
        with (
            tc.tile_pool(name="const", bufs=1) as cp,
            tc.tile_pool(name="gbig", bufs=1) as gp,
            tc.tile_pool(name="stage", bufs=2) as sp,
            tc.tile_pool(name="state", bufs=1) as st,
            tc.tile_pool(name="small", bufs=2) as sm,
            tc.tile_pool(name="dram", bufs=1, space="DRAM") as dp,
            tc.tile_pool(name="psum", bufs=2, space="PSUM") as pp,
        ):
            # ---- constants: A = exp(T) bf16 (natural [j,k]) and its transpose
            t_f32 = cp.tile([C, C], f32, tag="t_f32")
            nc.sync.dma_start(t_f32[:], Tm[:])
            AB = cp.tile([C, C], bf16, tag="AB")
            nc.scalar.activation(AB[:], t_f32[:], mybir.ActivationFunctionType.Exp)
            AF = cp.tile([C, C], bf16, tag="AF")
            nc.sync.dma_start_transpose(AF[:], AB[:])
            ones_col = cp.tile([C, 1], bf16, tag="ones_col")
            nc.vector.memset(ones_col[:], 1.0)
            ones_row = cp.tile([1, C], bf16, tag="ones_row")
            nc.vector.memset(ones_row[:], 1.0)
            biasc = cp.tile([128, 1], f32, tag="biasc")
            nc.vector.memset(biasc[:], -C0)

            # ---- gold setup: iota + transposed tags (all DVE/DMA, no gpsimd
            # in the hot loop; the two setup iotas are gpsimd but tiny)
            iota_i = cp.tile([C, C], i32, tag="iota_i")
            nc.gpsimd.iota(iota_i[:], pattern=[[1, C]], base=0, channel_multiplier=0)
            iota_bf = cp.tile([C, C], bf16, tag="iota_bf")
            nc.vector.tensor_copy(iota_bf[:], iota_i[:])
            # tags -> bf16 -> DRAM -> xbar transpose: tgT[p, b*8+k] = tags[b, k*128+p]
            tg = cp.tile([BC, L], i32, tag="tg")
            nc.sync.dma_start(tg[:], tags[:])
            tg_bf = cp.tile([BC, L], bf16, tag="tg_bf")
            nc.vector.tensor_copy(tg_bf[:], tg[:])
            tscr = dp.tile([C + BC * L, 1], bf16, tag="tscr")   # 128-elem pad at front
            nc.sync.dma_start(
                tscr[C:C + BC * L, 0:1].rearrange("(b t) o -> b (t o)", b=BC), tg_bf[:])
            tgT_bf = cp.tile([C, BC * NTCH], bf16, tag="tgT_bf")
            nc.sync.dma_start_transpose(
                tgT_bf[:], tscr[C:C + BC * L, 0:1].rearrange("(m p) o -> m (p o)", p=C))
            tgTs_bf = cp.tile([C, BC * NTCH], bf16, tag="tgTs_bf")
            nc.sync.dma_start_transpose(
                tgTs_bf[:], tscr[C - 1:C - 1 + BC * L, 0:1].rearrange("(m p) o -> m (p o)", p=C))
            # held PSUM accumulator for transition counts
            tracc = pp.tile([C, C], f32, name="tracc", tag="tracc", bufs=1)
            emit_acc = cp.tile([C, NCH + 1], f32, tag="emit_acc")

            # ---- stream feats -> exp -> t-major bf16 scratch -> transpose to SBUF
            # scratch row r = t*BC + b;  g columns land as [c, t*BC + b]
            gnat = dp.tile([L * BC, C], bf16, tag="gnat")
            g = gp.tile([C, L * BC], bf16, tag="g")      # 64KB/partition
            RPC = TCH * BC                  # rows per chunk (4096)
            gnat_r = gnat.rearrange("(p w) c -> p (w c)", w=BC)
            order = []                      # serve both chain ends first
            lo, hi = 0, NCH - 1
            while lo <= hi:
                order.append(lo)
                if hi != lo:
                    order.append(hi)
                lo, hi = lo + 1, hi - 1
            nmm = [0]
            for k in order:
                s_in = sp.tile([128, BC * C], f32, name="s_in", tag="s_in")
                src = feats[:, k * TCH:(k + 1) * TCH, :].rearrange("b tt c -> tt b c")
                nc.sync.dma_start(s_in[:].rearrange("p (b c) -> p b c", b=BC), src)
                s_bf = sp.tile([128, BC * C], bf16, name="s_bf", tag="s_bf")
                nc.scalar.activation(s_bf[:], s_in[:],
                                     mybir.ActivationFunctionType.Exp, bias=biasc[:])
                nc.sync.dma_start(gnat_r[k * 128:(k + 1) * 128, :], s_bf[:])
                nc.sync.dma_start_transpose(
                    g[:, k * RPC:(k + 1) * RPC], gnat[k * RPC:(k + 1) * RPC, :])

                # ---- gold contributions for this chunk (bulk, no gpsimd)
                # ohc[p, b*128+lab] = (lab == tags[b, k*128+p])
                ohc = sp.tile([C, BC * C], bf16, name="ohc", tag="ohc")
                i0 = iota_bf[:].rearrange("p (o l) -> p o l", o=1)\
                    .broadcast_to([C, BC, C])
                cur = tgT_bf[:].rearrange("p (b r) -> p b r", r=NTCH)[:, :, k:k + 1]
                ohc_v = ohc[:].rearrange("p (b l) -> p b l", b=BC)
                nc.vector.tensor_tensor(out=ohc_v, in0=i0,
                                        in1=cur.broadcast_to([C, BC, C]), op=EQ)
                ohp = sp.tile([C, BC * C], bf16, name="ohp", tag="ohp")
                prv = tgTs_bf[:].rearrange("p (b r) -> p b r", r=NTCH)[:, :, k:k + 1]
                ohp_v = ohp[:].rearrange("p (b l) -> p b l", b=BC)
                nc.vector.tensor_tensor(out=ohp_v, in0=i0,
                                        in1=prv.broadcast_to([C, BC, C]), op=EQ)
                if k == 0:
                    nc.vector.memset(ohp[0:1, :], 0.0)   # t=0 has no predecessor
                # emissions: emit_acc[:, k] = sum_f (ohc * feats_chunk)
                ev = sp.tile([C, BC * C], bf16, name="ev", tag="ev")
                nc.vector.scalar_tensor_tensor(
                    out=ev[:], in0=ohc[:], scalar=1.0, in1=s_in[:],
                    op0=MUL, op1=MUL, accum_out=emit_acc[:, k:k + 1])
                # transition counts: tracc += ohc_b^T @ ohp_b
                for b in range(BC):
                    first = nmm[0] == 0
                    last = nmm[0] == BC * NCH - 1
                    nc.tensor.matmul(tracc[:], ohc[:, b * C:(b + 1) * C],
                                     ohp[:, b * C:(b + 1) * C],
                                     start=first, stop=last, skip_group_check=True)
                    nmm[0] += 1

            # ---- the two chains
            EBUF = 3
            Ebufs = [st.tile([C, BC], bf16, name=f"E{i}", tag=f"E{i}") for i in range(EBUF)]
            Vbufs = [st.tile([C, BC], bf16, name=f"V{i}", tag=f"V{i}") for i in range(EBUF)]
            nc.vector.tensor_copy(Ebufs[0][:], g[:, 0:BC])                     # alpha_0
            nc.vector.tensor_copy(Vbufs[0][:], g[:, (L - 1) * BC: L * BC])     # v_{L-1}

            fwd_gs = {}
            bwd_gs = {}

            def renorm_prep(src_state, gslice, rec_row, which):
                """Compute rec = 1/colsum(state), record it, scale the g slice."""
                cs = pp.tile([1, BC], f32, name="cs", tag="cs", bufs=1)
                nc.tensor.matmul(cs[:], ones_col[:], src_state[:], start=True, stop=True)
                rec = sm.tile([1, BC], bf16, name="rec_bf", tag="rec_bf")
                with nc.allow_low_precision(reason="applied scale is recorded exactly"):
                    nc.vector.reciprocal(rec[:], cs[:])
                rec_st = sm.tile([1, BC], f32, name="rec_st", tag="rec_st")
                nc.scalar.copy(rec_st[:], rec[:])
                nc.sync.dma_start(rec_row, rec_st[:])
                bc = pp.tile([C, BC], f32, name="bc", tag="bc", bufs=1)
                nc.tensor.matmul(bc[:], ones_row[:], rec[:], start=True, stop=True)
                gs = sm.tile([C, BC], bf16, name=f"gs{which}", tag=f"gs{which}")
                nc.vector.tensor_tensor(out=gs[:], in0=bc[:], in1=gslice, op=MUL)
                return gs

            fi = bi = 0
            for i in range(511):
                tf = i + 1                       # fwd round: produces alpha-state tf
                tb = L - 1 - i                   # bwd round: consumes g_{tb-1}
                if tf + RLAG in FWD_REN:
                    tr = tf + RLAG
                    fwd_gs[tr] = renorm_prep(Ebufs[i % EBUF],
                                             g[:, tr * BC:(tr + 1) * BC],
                                             recf_o[fi:fi + 1, :], "f")
                    fi += 1
                if tb - RLAG in BWD_REN:
                    tr = tb - RLAG
                    bwd_gs[tr] = renorm_prep(Vbufs[i % EBUF],
                                             g[:, (tr - 1) * BC: tr * BC],
                                             recb_o[bi:bi + 1, :], "b")
                    bi += 1
                psf = pp.tile([C, BC], f32, name="psf", tag="psf", bufs=2)
                nc.tensor.matmul(psf[:], AF[:], Ebufs[i % EBUF][:], start=True, stop=True)
                gin = fwd_gs.pop(tf, None)
                gsl = gin[:] if gin is not None else g[:, tf * BC:(tf + 1) * BC]
                nc.vector.tensor_tensor(out=Ebufs[(i + 1) % EBUF][:], in0=psf[:],
                                        in1=gsl, op=MUL)
                psb = pp.tile([C, BC], f32, name="psb", tag="psb", bufs=2)
                nc.tensor.matmul(psb[:], AB[:], Vbufs[i % EBUF][:], start=True, stop=True)
                gin = bwd_gs.pop(tb, None)
                gsl = gin[:] if gin is not None else g[:, (tb - 1) * BC: tb * BC]
                nc.vector.tensor_tensor(out=Vbufs[(i + 1) % EBUF][:], in0=psb[:],
                                        in1=gsl, op=MUL)

            # fwd state after 511 rounds = alpha~_511; bwd needs one more matmul
            ef_t = st.tile([C, BC], f32, name="ef_t", tag="ef_t")
            nc.scalar.copy(ef_t[:], Ebufs[511 % EBUF][:])
            nc.sync.dma_start(ef_o[:], ef_t[:])
            psb = pp.tile([C, BC], f32, name="psb_fin", tag="psb", bufs=2)
            nc.tensor.matmul(psb[:], AB[:], Vbufs[511 % EBUF][:], start=True, stop=True)
            eb_t = st.tile([C, BC], f32, name="eb_t", tag="eb_t")
            nc.scalar.copy(eb_t[:], psb[:])
            nc.sync.dma_start(eb_o[:], eb_t[:])
            # gold extraction: trans = <T, tracc>, emissions already in emit_acc
            scr1 = cp.tile([C, C], f32, tag="scr1")
            nc.vector.scalar_tensor_tensor(
                out=scr1[:], in0=tracc[:], scalar=1.0, in1=t_f32[:],
                op0=MUL, op1=MUL, accum_out=emit_acc[:, NCH:NCH + 1])
            nc.sync.dma_start(gold_o[:], emit_acc[:])

    nc.compile()
    return nc


def _get_nc():
    if "nc" not in _cache:
        _cache["nc"] = _build()
    return _cache["nc"]


def kernel(feats, tags, T, _trace=False, _trace_kwargs=None):
    from concourse.bass_utils import run_bass_kernel_spmd

    feats = np.ascontiguousarray(feats, dtype=np.float32)
    tags = np.ascontiguousarray(tags, dtype=np.int32)
    T = np.ascontiguousarray(T, dtype=np.float32)

    nc = _get_nc()
    in_maps = []
    for c in range(NCORES):
        sl = slice(c * BC, (c + 1) * BC)
        in_maps.append({"feats": feats[sl], "tags": tags[sl], "T": T})
    res = run_bass_kernel_spmd(nc, in_maps, core_ids=list(range(NCORES)),
                               trace=_trace, **(_trace_kwargs or {}))
    if _trace:
        _cache["last_results"] = res

    logZ = np.zeros(B)
    gold_total = 0.0
    for c, r in enumerate(res.results):
        sl = slice(c * BC, (c + 1) * BC)
        ef = r["ef"].astype(np.float64)
        eb = r["eb"].astype(np.float64)
        carry = 0.0
        if NRF:
            carry = carry - np.log(r["recf"].astype(np.float64)).sum(axis=0)
        if NRB:
            carry = carry - np.log(r["recb"].astype(np.float64)).sum(axis=0)
        logZ[sl] = np.log((ef * eb).sum(axis=0)) + carry + L * C0
        gold_total += float(r["gold"].astype(np.float64).sum())
    loss = logZ.mean() - gold_total / B
    return np.float32(loss)
